# revision 6
# baseline (speedup 1.0000x reference)
"""BAP classifier (attention-pooling + linear head) on 8 TRN2 NeuronCores.

Pipeline (reference math):
    A    = sigmoid(einsum('bchw,mc->bmhw', x, Wa) + ba)     # attention maps
    bap  = einsum('bchw,bmhw->bmc', x, A) / (H*W)           # attn-weighted pool
    out  = bap.reshape(B, M*C) @ Wc.T + bc                  # linear head

Sharding:
  Phase 1 — data-parallel over batch (8 batches/core): each core computes
    raw feats rows [8, M*C] (un-normalized bap, transposed per batch on chip).
  Phase 2 — Wc column-parallel (8192 columns of the M*C dim per core): each
    core computes a partial [B, NCLS] logit; host sums partials, applies the
    1/(H*W) scale and bias.
"""
import sys

if "/opt/trn_rl_repo" not in sys.path:
    sys.path.insert(0, "/opt/trn_rl_repo")

import numpy as np

import concourse.bacc as bacc
import concourse.mybir as mybir
from concourse.tile import TileContext
from concourse.bass_utils import run_bass_kernel_spmd
from concourse.masks import make_identity

B, C, H, W = 64, 2048, 14, 14
HW = H * W                     # 196
M, NCLS = 32, 396
NCORES = 8
BPC = B // NCORES              # 8 batches per core
CT = C // 128                  # 16 c-chunks
KTOT = M * C                   # 65536
KPC = KTOT // NCORES           # 8192 Wc columns per core
KT = KPC // 128                # 64 k-tiles per core in phase 2

F32 = mybir.dt.float32
F32R = mybir.dt.float32r
BF16 = mybir.dt.bfloat16

# Run options (test harness may flip these; defaults are what grading uses).
TRACE = False
TRACE_INFO = {}

_cache = {}


def _nc():
    return bacc.Bacc(
        "TRN2", target_bir_lowering=False, debug=False, num_devices=NCORES
    )


def _build_phase1():
    """Per-core: x_shard (bf16) [BPC, C, HW] -> raw feats [BPC, M*C] (no 1/HW scale)."""
    nc = _nc()
    x = nc.dram_tensor("x", [BPC, C, HW], BF16, kind="ExternalInput")
    wat = nc.dram_tensor("wat", [C, M], BF16, kind="ExternalInput")
    ba = nc.dram_tensor("ba", [M, 1], F32, kind="ExternalInput")
    feats = nc.dram_tensor("feats", [BPC, M * C], F32, kind="ExternalOutput")

    with TileContext(nc) as tc:
        with (
            tc.tile_pool(name="const", bufs=1) as const,
            tc.tile_pool(name="xpool", bufs=2) as xpool,
            tc.tile_pool(name="xtpool", bufs=2) as xtpool,
            tc.tile_pool(name="apool", bufs=2) as apool,
            tc.tile_pool(name="atpool", bufs=2) as atpool,
            tc.tile_pool(name="fpool", bufs=2) as fpool,
            tc.tile_pool(name="ps_att", bufs=2, space="PSUM") as ps_att,
            tc.tile_pool(name="ps_tr", bufs=2, space="PSUM") as ps_tr,
            tc.tile_pool(name="ps_bap", bufs=3, space="PSUM") as ps_bap,
        ):
            wat_sb = const.tile([128, CT, M], BF16)
            nc.sync.dma_start(
                out=wat_sb, in_=wat.ap().rearrange("(t p) m -> p t m", p=128)
            )
            ba_sb = const.tile([M, 1], F32)
            nc.sync.dma_start(out=ba_sb, in_=ba.ap())
            ident = const.tile([M, M], BF16)
            make_identity(nc, ident)

            for pr in range(BPC // 2):
                # [c-part, batch-in-pair, c-chunk, hw]
                x_pair = xpool.tile([128, 2, CT, HW], BF16)
                nc.sync.dma_start(
                    out=x_pair,
                    in_=x.ap()[2 * pr : 2 * pr + 2].rearrange(
                        "b (t p) f -> p b t f", p=128
                    ),
                )

                # einsum1: att[m, (b2 hw)] = sum_c Wa[m,c] * x[c, (b2 hw)]
                att_ps = ps_att.tile([M, 2, HW], F32)
                for ct in range(CT):
                    nc.tensor.matmul(
                        att_ps,
                        lhsT=wat_sb[:, ct, :],
                        rhs=x_pair[:, :, ct, :],
                        start=(ct == 0),
                        stop=(ct == CT - 1),
                    )
                a_sb = apool.tile([M, 2, HW], BF16)
                nc.scalar.activation(
                    out=a_sb,
                    in_=att_ps,
                    func=mybir.ActivationFunctionType.Sigmoid,
                    bias=ba_sb,
                )

                for b2 in range(2):
                    b = 2 * pr + b2
                    # x^T via DMA xbar transpose straight from DRAM (bf16):
                    # xta[hw, c] for hw 0..127, xtb[hw, c] for hw 68..195.
                    xta = xtpool.tile([128, C], BF16, tag="xta")
                    xtb = xtpool.tile([128, C], BF16, tag="xtb")
                    nc.sync.dma_start(out=xta, in_=x.ap()[b, :, 0:128], transpose=True)
                    nc.sync.dma_start(out=xtb, in_=x.ap()[b, :, 68:196], transpose=True)

                    # A^T chunks via PE transpose
                    ata_ps = ps_tr.tile([128, M], BF16, tag="ata")
                    nc.tensor.transpose(
                        ata_ps, a_sb[:, b2, 0:128], ident[0:M, 0:M]
                    )
                    ata = atpool.tile([128, M], BF16, tag="ata_sb")
                    nc.scalar.copy(out=ata, in_=ata_ps)

                    # hw 68..195 at base partition 0; rows 0..59 (hw 68..127
                    # overlap with chunk A) are zeroed so they contribute 0.
                    atb_ps = ps_tr.tile([128, M], BF16, tag="ata")
                    nc.tensor.transpose(
                        atb_ps, a_sb[:, b2, 68:196], ident[0:M, 0:M]
                    )
                    atb = atpool.tile([128, M], BF16, tag="atb_sb")
                    nc.scalar.copy(out=atb, in_=atb_ps)
                    nc.vector.memset(atb[0:60, :], 0.0)

                    # einsum2: bapT[m, c] = sum_hw A[m,hw] * x[c,hw]
                    featsb = fpool.tile([M, C], F32)
                    for nt in range(4):
                        bap_ps = ps_bap.tile([M, 512], F32)
                        nc.tensor.matmul(
                            bap_ps,
                            lhsT=ata,
                            rhs=xta[:, 512 * nt : 512 * (nt + 1)],
                            start=True,
                            stop=False,
                        )
                        nc.tensor.matmul(
                            bap_ps,
                            lhsT=atb,
                            rhs=xtb[:, 512 * nt : 512 * (nt + 1)],
                            start=False,
                            stop=True,
                        )
                        nc.scalar.copy(
                            out=featsb[:, 512 * nt : 512 * (nt + 1)], in_=bap_ps
                        )
                    nc.sync.dma_start(
                        out=feats.ap()[b].rearrange("(m c) -> m c", m=M),
                        in_=featsb,
                    )
    nc.compile()
    return nc


def _build_phase2():
    """Per-core: featsT slice [KPC, B] x WcT slice [KPC, NCLS] -> partial [B, NCLS]."""
    nc = _nc()
    ft = nc.dram_tensor("ft", [KPC, B], F32R, kind="ExternalInput")
    wct = nc.dram_tensor("wct", [KPC, NCLS], F32R, kind="ExternalInput")
    part = nc.dram_tensor("part", [B, NCLS], F32, kind="ExternalOutput")

    KCH = 8  # k-tiles per DMA chunk

    with TileContext(nc) as tc:
        with (
            tc.tile_pool(name="fpool", bufs=1) as fpool,
            tc.tile_pool(name="wpool", bufs=3) as wpool,
            tc.tile_pool(name="opool", bufs=1) as opool,
            tc.tile_pool(name="ps_out", bufs=1, space="PSUM") as ps_out,
        ):
            ft_sb = fpool.tile([128, KT, B], F32R)
            nc.sync.dma_start(
                out=ft_sb, in_=ft.ap().rearrange("(t p) b -> p t b", p=128)
            )
            out_ps = ps_out.tile([B, NCLS], F32)
            for kc in range(KT // KCH):
                w_sb = wpool.tile([128, KCH, NCLS], F32R)
                nc.sync.dma_start(
                    out=w_sb,
                    in_=wct.ap()[128 * KCH * kc : 128 * KCH * (kc + 1)].rearrange(
                        "(t p) n -> p t n", p=128
                    ),
                )
                for kl in range(KCH):
                    kt = kc * KCH + kl
                    nc.tensor.matmul(
                        out_ps,
                        lhsT=ft_sb[:, kt, :],
                        rhs=w_sb[:, kl, :],
                        start=(kt == 0),
                        stop=(kt == KT - 1),
                    )
            out_sb = opool.tile([B, NCLS], F32)
            nc.scalar.copy(out=out_sb, in_=out_ps)
            nc.sync.dma_start(out=part.ap(), in_=out_sb)
    nc.compile()
    return nc


def _install_ntff_hook():
    import types

    import trn_agent_boot.trn_boot as tb
    import concourse.bass_utils as bu

    hook = tb._ntff_profile_via_ctypes("/opt/axon/libaxon_pjrt.so")
    mod = types.ModuleType("antenv.axon_hooks")
    mod.get_axon_ntff_profile_hook = lambda: hook
    sys.modules["antenv.axon_hooks"] = mod
    bu.upload_artifacts = lambda tmpdir: "(skipped)"


def _run(nc, in_maps, label):
    core_ids = list(range(NCORES))
    if TRACE:
        _install_ntff_hook()
        res = run_bass_kernel_spmd(nc, in_maps, core_ids, trace=True)
        TRACE_INFO[label] = res.exec_time_ns
    else:
        res = run_bass_kernel_spmd(nc, in_maps, core_ids)
    return res.results


def kernel(x, Wa, ba, Wc, bc):
    import ml_dtypes

    bf16 = np.dtype(ml_dtypes.bfloat16)
    x = np.ascontiguousarray(x, dtype=np.float32).reshape(B, C, HW).astype(bf16)
    wat = np.ascontiguousarray(Wa.T, dtype=np.float32).astype(bf16)
    ba2 = np.ascontiguousarray(ba, dtype=np.float32).reshape(M, 1)
    wct = np.ascontiguousarray(Wc.T, dtype=np.float32)  # [KTOT, NCLS]

    if "p1" not in _cache:
        _cache["p1"] = _build_phase1()
    if "p2" not in _cache:
        _cache["p2"] = _build_phase2()

    in1 = [
        {"x": x[i * BPC : (i + 1) * BPC], "wat": wat, "ba": ba2}
        for i in range(NCORES)
    ]
    res1 = _run(_cache["p1"], in1, "phase1")
    feats = np.concatenate([r["feats"] for r in res1], axis=0)  # [B, KTOT]

    featsT = np.ascontiguousarray(feats.T)  # [KTOT, B]
    in2 = [
        {
            "ft": featsT[i * KPC : (i + 1) * KPC],
            "wct": wct[i * KPC : (i + 1) * KPC],
        }
        for i in range(NCORES)
    ]
    res2 = _run(_cache["p2"], in2, "phase2")
    parts = np.stack([r["part"] for r in res2], axis=0)  # [NCORES, B, NCLS]

    logits = parts.sum(axis=0) / float(HW) + np.asarray(bc, dtype=np.float32)
    return logits.astype(np.float32)


# revision 11
# speedup vs baseline: 1.3428x; 1.3428x over previous
"""BAP classifier (attention-pooling + linear head) on 8 TRN2 NeuronCores.

Pipeline (reference math):
    A    = sigmoid(einsum('bchw,mc->bmhw', x, Wa) + ba)     # attention maps
    bap  = einsum('bchw,bmhw->bmc', x, A) / (H*W)           # attn-weighted pool
    out  = bap.reshape(B, M*C) @ Wc.T + bc                  # linear head

Sharding:
  Phase 1 — data-parallel over batch (8 batches/core): each core computes
    raw feats rows [8, M*C] (un-normalized bap, transposed per batch on chip).
  Phase 2 — Wc column-parallel (8192 columns of the M*C dim per core): each
    core computes a partial [B, NCLS] logit; host sums partials, applies the
    1/(H*W) scale and bias.

Compute dtype is bf16 on the TensorEngine with fp32 PSUM accumulation
(rel err vs the fp32 reference lands ~3e-3).
"""
import sys

if "/opt/trn_rl_repo" not in sys.path:
    sys.path.insert(0, "/opt/trn_rl_repo")

import numpy as np

import concourse.bacc as bacc
import concourse.mybir as mybir
from concourse.tile import TileContext
from concourse.bass_utils import run_bass_kernel_spmd
from concourse.masks import make_identity

B, C, H, W = 64, 2048, 14, 14
HW = H * W                     # 196
M, NCLS = 32, 396
NCORES = 8
BPC = B // NCORES              # 8 batches per core
CT = C // 128                  # 16 c-chunks
KTOT = M * C                   # 65536
KPC = KTOT // NCORES           # 8192 Wc columns per core
KT = KPC // 128                # 64 k-tiles per core in phase 2

F32 = mybir.dt.float32
F32R = mybir.dt.float32r
BF16 = mybir.dt.bfloat16

# Run options (test harness may flip these; defaults are what grading uses).
TRACE = False
TRACE_INFO = {}
TRACE_RES = {}

_cache = {}


def _nc():
    return bacc.Bacc(
        "TRN2", target_bir_lowering=False, debug=False, num_devices=NCORES
    )


def _build_phase1():
    """Per-core: x_shard (bf16) [BPC, C, HW] -> raw feats [BPC, M*C].

    c is loaded with the permuted mapping c = p*CT + t (p = partition,
    t = chunk) so every natural-load descriptor is one contiguous 6.1KB run;
    wat arrives host-permuted to the same mapping.  x^T for the BAP einsum
    comes from two full-height DRAM xbar transposes per batch (hw 0:128 and
    68:196, with the overlapping A^T rows zeroed).
    """
    nc = _nc()
    x = nc.dram_tensor("x", [BPC, C, HW], BF16, kind="ExternalInput")
    wat = nc.dram_tensor("wat", [128, CT, M], BF16, kind="ExternalInput")
    ba = nc.dram_tensor("ba", [M, 1], F32, kind="ExternalInput")
    feats = nc.dram_tensor("feats", [BPC, M * C], F32, kind="ExternalOutput")

    with TileContext(nc) as tc:
        with (
            tc.tile_pool(name="const", bufs=1) as const,
            tc.tile_pool(name="xpool", bufs=2) as xpool,
            tc.tile_pool(name="xtpool", bufs=2) as xtpool,
            tc.tile_pool(name="apool", bufs=2) as apool,
            tc.tile_pool(name="atpool", bufs=2) as atpool,
            tc.tile_pool(name="fpool", bufs=2) as fpool,
            tc.tile_pool(name="ps_att", bufs=1, space="PSUM") as ps_att,
            tc.tile_pool(name="ps_tr", bufs=2, space="PSUM") as ps_tr,
            tc.tile_pool(name="ps_bap", bufs=4, space="PSUM") as ps_bap,
        ):
            wat_sb = const.tile([128, CT, M], BF16)
            nc.sync.dma_start(out=wat_sb, in_=wat.ap())
            ba_sb = const.tile([M, 1], F32)
            nc.sync.dma_start(out=ba_sb, in_=ba.ap())
            ident = const.tile([M, M], BF16)
            make_identity(nc, ident)

            for q in range(BPC // 4):
                # [c-part, batch-in-quad, c-chunk, hw] with c = p*CT + t
                x_quad = xpool.tile([128, 4, CT, HW], BF16)
                nc.scalar.dma_start(
                    out=x_quad,
                    in_=x.ap()[4 * q : 4 * q + 4].rearrange(
                        "b (p t) f -> p b t f", t=CT
                    ),
                )

                # einsum1: att[m, (b4 hw)] = sum_c Wa[m,c] * x[c, (b4 hw)]
                # (two matmuls per c-chunk: a single fp32 matmul output must
                # stay within one 2KB PSUM bank, i.e. N <= 512)
                att01 = ps_att.tile([M, 2, HW], F32, tag="att01")
                att23 = ps_att.tile([M, 2, HW], F32, tag="att23")
                for ct in range(CT):
                    nc.tensor.matmul(
                        att01,
                        lhsT=wat_sb[:, ct, :],
                        rhs=x_quad[:, 0:2, ct, :],
                        start=(ct == 0),
                        stop=(ct == CT - 1),
                    )
                    nc.tensor.matmul(
                        att23,
                        lhsT=wat_sb[:, ct, :],
                        rhs=x_quad[:, 2:4, ct, :],
                        start=(ct == 0),
                        stop=(ct == CT - 1),
                    )
                a_sb = apool.tile([M, 4, HW], BF16)
                nc.scalar.activation(
                    out=a_sb[:, 0:2, :],
                    in_=att01,
                    func=mybir.ActivationFunctionType.Sigmoid,
                    bias=ba_sb,
                )
                nc.scalar.activation(
                    out=a_sb[:, 2:4, :],
                    in_=att23,
                    func=mybir.ActivationFunctionType.Sigmoid,
                    bias=ba_sb,
                )

                # feats staging for the whole quad: partition = 32*b4 + m
                featsq = fpool.tile([128, C], F32)

                for b4 in range(4):
                    b = 4 * q + b4
                    # x^T via DMA xbar transpose straight from DRAM (bf16):
                    # xta[hw, c] for hw 0..127, xtb[hw, c] for hw 68..195.
                    xta = xtpool.tile([128, C], BF16, tag="xta")
                    xtb = xtpool.tile([128, C], BF16, tag="xtb")
                    nc.sync.dma_start(
                        out=xta, in_=x.ap()[b, :, 0:128], transpose=True
                    )
                    nc.sync.dma_start(
                        out=xtb, in_=x.ap()[b, :, 68:196], transpose=True
                    )

                    # A^T chunks via PE transpose
                    ata_ps = ps_tr.tile([128, M], BF16, tag="ata")
                    nc.tensor.transpose(
                        ata_ps, a_sb[:, b4, 0:128], ident[0:M, 0:M]
                    )
                    ata = atpool.tile([128, M], BF16, tag="ata_sb")
                    nc.scalar.copy(out=ata, in_=ata_ps)

                    # hw 68..195 at base partition 0; rows 0..59 (hw 68..127
                    # overlap with chunk A) are zeroed so they contribute 0.
                    atb_ps = ps_tr.tile([128, M], BF16, tag="ata")
                    nc.tensor.transpose(
                        atb_ps, a_sb[:, b4, 68:196], ident[0:M, 0:M]
                    )
                    atb = atpool.tile([128, M], BF16, tag="atb_sb")
                    nc.scalar.copy(out=atb, in_=atb_ps)
                    nc.vector.memset(atb[0:60, :], 0.0)

                    # einsum2: bapT[m, c] = sum_hw A[m,hw] * x[c,hw]
                    # All four 512-wide output chunks per A^T half so the
                    # PE loads each stationary operand once.
                    bap_ps = [
                        ps_bap.tile([M, 512], F32, tag="bap", name=f"bap_ps{nt}")
                        for nt in range(4)
                    ]
                    for nt in range(4):
                        nc.tensor.matmul(
                            bap_ps[nt],
                            lhsT=ata,
                            rhs=xta[:, 512 * nt : 512 * (nt + 1)],
                            start=True,
                            stop=False,
                        )
                    for nt in range(4):
                        nc.tensor.matmul(
                            bap_ps[nt],
                            lhsT=atb,
                            rhs=xtb[:, 512 * nt : 512 * (nt + 1)],
                            start=False,
                            stop=True,
                        )
                    for nt in range(4):
                        nc.vector.tensor_copy(
                            out=featsq[
                                32 * b4 : 32 * (b4 + 1),
                                512 * nt : 512 * (nt + 1),
                            ],
                            in_=bap_ps[nt],
                        )
                nc.gpsimd.dma_start(
                    out=feats.ap()[4 * q : 4 * q + 4].rearrange(
                        "b (m c) -> (b m) c", m=M
                    ),
                    in_=featsq,
                )
    nc.compile()
    return nc


def _build_phase2():
    """Per-core: featsT slice (partition-major, bf16) x WcT slice (bf16)
    -> partial [B, NCLS] (fp32)."""
    nc = _nc()
    ft = nc.dram_tensor("ft", [128, KT, B], BF16, kind="ExternalInput")
    wct = nc.dram_tensor("wct", [KPC, NCLS], BF16, kind="ExternalInput")
    part = nc.dram_tensor("part", [B, NCLS], F32, kind="ExternalOutput")

    KCH = 4  # k-tiles per DMA chunk

    with TileContext(nc) as tc:
        with (
            tc.tile_pool(name="fpool", bufs=1) as fpool,
            tc.tile_pool(name="wpool", bufs=6) as wpool,
            tc.tile_pool(name="opool", bufs=1) as opool,
            tc.tile_pool(name="ps_out", bufs=1, space="PSUM") as ps_out,
        ):
            ft_sb = fpool.tile([128, KT, B], BF16)
            nc.sync.dma_start(out=ft_sb, in_=ft.ap())
            out_ps = ps_out.tile([B, NCLS], F32)
            for kc in range(KT // KCH):
                w_sb = wpool.tile([128, KCH, NCLS], BF16)
                eng = nc.scalar if kc % 2 else nc.sync
                eng.dma_start(
                    out=w_sb,
                    in_=wct.ap()[128 * KCH * kc : 128 * KCH * (kc + 1)].rearrange(
                        "(t p) n -> p t n", p=128
                    ),
                )
                for kl in range(KCH):
                    kt = kc * KCH + kl
                    nc.tensor.matmul(
                        out_ps,
                        lhsT=ft_sb[:, kt, :],
                        rhs=w_sb[:, kl, :],
                        start=(kt == 0),
                        stop=(kt == KT - 1),
                    )
            out_sb = opool.tile([B, NCLS], F32)
            nc.scalar.copy(out=out_sb, in_=out_ps)
            nc.sync.dma_start(out=part.ap(), in_=out_sb)
    nc.compile()
    return nc


def _install_ntff_hook():
    import types

    import trn_agent_boot.trn_boot as tb
    import concourse.bass_utils as bu

    hook = tb._ntff_profile_via_ctypes("/opt/axon/libaxon_pjrt.so")
    mod = types.ModuleType("antenv.axon_hooks")
    mod.get_axon_ntff_profile_hook = lambda: hook
    sys.modules["antenv.axon_hooks"] = mod
    bu.upload_artifacts = lambda tmpdir: "(skipped)"


def _run(nc, in_maps, label):
    core_ids = list(range(NCORES))
    if TRACE:
        _install_ntff_hook()
        res = run_bass_kernel_spmd(nc, in_maps, core_ids, trace=True)
        TRACE_INFO[label] = res.exec_time_ns
        TRACE_RES[label] = res
    else:
        res = run_bass_kernel_spmd(nc, in_maps, core_ids)
    return res.results


def kernel(x, Wa, ba, Wc, bc):
    import ml_dtypes

    bf16 = np.dtype(ml_dtypes.bfloat16)
    x = np.ascontiguousarray(x, dtype=np.float32).reshape(B, C, HW).astype(bf16)
    # wat[p, t, m] = Wa[m, p*CT + t] — matches the kernel's permuted c layout
    wat = np.ascontiguousarray(Wa.T, dtype=np.float32).astype(bf16).reshape(
        128, CT, M
    )
    ba2 = np.ascontiguousarray(ba, dtype=np.float32).reshape(M, 1)
    wct = np.ascontiguousarray(Wc.T, dtype=np.float32).astype(bf16)  # [KTOT, NCLS]

    if "p1" not in _cache:
        _cache["p1"] = _build_phase1()
    if "p2" not in _cache:
        _cache["p2"] = _build_phase2()

    in1 = [
        {"x": x[i * BPC : (i + 1) * BPC], "wat": wat, "ba": ba2}
        for i in range(NCORES)
    ]
    res1 = _run(_cache["p1"], in1, "phase1")
    feats = np.concatenate([r["feats"] for r in res1], axis=0)  # [B, KTOT] f32

    # ft[p, t, b] = feats[b, kslice + t*128 + p] (partition-major, bf16)
    featsT = np.ascontiguousarray(feats.T).astype(bf16)  # [KTOT, B]
    in2 = [
        {
            "ft": np.ascontiguousarray(
                featsT[i * KPC : (i + 1) * KPC].reshape(KT, 128, B).transpose(
                    1, 0, 2
                )
            ),
            "wct": wct[i * KPC : (i + 1) * KPC],
        }
        for i in range(NCORES)
    ]
    res2 = _run(_cache["p2"], in2, "phase2")
    parts = np.stack([r["part"] for r in res2], axis=0)  # [NCORES, B, NCLS]

    logits = parts.sum(axis=0) / float(HW) + np.asarray(bc, dtype=np.float32)
    return logits.astype(np.float32)


# revision 12
# speedup vs baseline: 1.4235x; 1.0601x over previous
"""BAP classifier (attention-pooling + linear head) on 8 TRN2 NeuronCores.

Pipeline (reference math):
    A    = sigmoid(einsum('bchw,mc->bmhw', x, Wa) + ba)     # attention maps
    bap  = einsum('bchw,bmhw->bmc', x, A) / (H*W)           # attn-weighted pool
    out  = bap.reshape(B, M*C) @ Wc.T + bc                  # linear head

Sharding:
  Phase 1 — data-parallel over batch (8 batches/core): each core computes
    raw feats rows [8, M*C] (un-normalized bap, transposed per batch on chip).
  Phase 2 — Wc column-parallel (8192 columns of the M*C dim per core): each
    core computes a partial [B, NCLS] logit; host sums partials, applies the
    1/(H*W) scale and bias.

Compute dtype is bf16 on the TensorEngine with fp32 PSUM accumulation
(rel err vs the fp32 reference lands ~3e-3).
"""
import sys

if "/opt/trn_rl_repo" not in sys.path:
    sys.path.insert(0, "/opt/trn_rl_repo")

import numpy as np

import concourse.bacc as bacc
import concourse.mybir as mybir
from concourse.tile import TileContext
from concourse.bass_utils import run_bass_kernel_spmd
from concourse.masks import make_identity

B, C, H, W = 64, 2048, 14, 14
HW = H * W                     # 196
M, NCLS = 32, 396
NCORES = 8
BPC = B // NCORES              # 8 batches per core
CT = C // 128                  # 16 c-chunks
KTOT = M * C                   # 65536
KPC = KTOT // NCORES           # 8192 Wc columns per core
KT = KPC // 128                # 64 k-tiles per core in phase 2

F32 = mybir.dt.float32
F32R = mybir.dt.float32r
BF16 = mybir.dt.bfloat16

# Run options (test harness may flip these; defaults are what grading uses).
TRACE = False
TRACE_INFO = {}
TRACE_RES = {}

_cache = {}


def _nc():
    return bacc.Bacc(
        "TRN2", target_bir_lowering=False, debug=False, num_devices=NCORES
    )


def _build_phase1():
    """Per-core: x_shard (bf16) [BPC, C, HW] -> raw feats [BPC, M*C].

    c is loaded with the permuted mapping c = p*CT + t (p = partition,
    t = chunk) so every natural-load descriptor is one contiguous 6.1KB run;
    wat arrives host-permuted to the same mapping.  x^T for the BAP einsum
    comes from two full-height DRAM xbar transposes per batch (hw 0:128 and
    68:196, with the overlapping A^T rows zeroed).
    """
    nc = _nc()
    x = nc.dram_tensor("x", [BPC, C, HW], BF16, kind="ExternalInput")
    wat = nc.dram_tensor("wat", [128, CT, M], BF16, kind="ExternalInput")
    ba = nc.dram_tensor("ba", [M, 1], F32, kind="ExternalInput")
    feats = nc.dram_tensor("feats", [BPC, M * C], F32, kind="ExternalOutput")

    with TileContext(nc) as tc:
        with (
            tc.tile_pool(name="const", bufs=1) as const,
            tc.tile_pool(name="xpool", bufs=2) as xpool,
            tc.tile_pool(name="xtpool", bufs=4) as xtpool,
            tc.tile_pool(name="apool", bufs=2) as apool,
            tc.tile_pool(name="atpool", bufs=4) as atpool,
            tc.tile_pool(name="fpool", bufs=2) as fpool,
            tc.tile_pool(name="ps_att", bufs=1, space="PSUM") as ps_att,
            tc.tile_pool(name="ps_tr", bufs=2, space="PSUM") as ps_tr,
            tc.tile_pool(name="ps_bap", bufs=4, space="PSUM") as ps_bap,
        ):
            wat_sb = const.tile([128, CT, M], BF16)
            nc.sync.dma_start(out=wat_sb, in_=wat.ap())
            ba_sb = const.tile([M, 1], F32)
            nc.sync.dma_start(out=ba_sb, in_=ba.ap())
            ident = const.tile([M, M], BF16)
            make_identity(nc, ident)

            for q in range(BPC // 4):
                # [c-part, batch-in-quad, c-chunk, hw] with c = p*CT + t
                x_quad = xpool.tile([128, 4, CT, HW], BF16)
                nc.scalar.dma_start(
                    out=x_quad,
                    in_=x.ap()[4 * q : 4 * q + 4].rearrange(
                        "b (p t) f -> p b t f", t=CT
                    ),
                )

                # einsum1: att[m, (b4 hw)] = sum_c Wa[m,c] * x[c, (b4 hw)]
                # (two matmuls per c-chunk: a single fp32 matmul output must
                # stay within one 2KB PSUM bank, i.e. N <= 512)
                att01 = ps_att.tile([M, 2, HW], F32, tag="att01")
                att23 = ps_att.tile([M, 2, HW], F32, tag="att23")
                for ct in range(CT):
                    nc.tensor.matmul(
                        att01,
                        lhsT=wat_sb[:, ct, :],
                        rhs=x_quad[:, 0:2, ct, :],
                        start=(ct == 0),
                        stop=(ct == CT - 1),
                    )
                    nc.tensor.matmul(
                        att23,
                        lhsT=wat_sb[:, ct, :],
                        rhs=x_quad[:, 2:4, ct, :],
                        start=(ct == 0),
                        stop=(ct == CT - 1),
                    )
                a_sb = apool.tile([M, 4, HW], BF16)
                nc.scalar.activation(
                    out=a_sb[:, 0:2, :],
                    in_=att01,
                    func=mybir.ActivationFunctionType.Sigmoid,
                    bias=ba_sb,
                )
                nc.scalar.activation(
                    out=a_sb[:, 2:4, :],
                    in_=att23,
                    func=mybir.ActivationFunctionType.Sigmoid,
                    bias=ba_sb,
                )

                # feats staging for the whole quad: partition = 32*b4 + m
                featsq = fpool.tile([128, C], F32)

                for b4 in range(4):
                    b = 4 * q + b4
                    # x^T via DMA xbar transpose straight from DRAM (bf16):
                    # xta[hw, c] for hw 0..127, xtb[hw, c] for hw 68..195.
                    xta = xtpool.tile([128, C], BF16, tag="xta")
                    xtb = xtpool.tile([128, C], BF16, tag="xtb")
                    nc.sync.dma_start(
                        out=xta, in_=x.ap()[b, :, 0:128], transpose=True
                    )
                    nc.sync.dma_start(
                        out=xtb, in_=x.ap()[b, :, 68:196], transpose=True
                    )

                    # A^T chunks via PE transpose
                    ata_ps = ps_tr.tile([128, M], BF16, tag="ata")
                    nc.tensor.transpose(
                        ata_ps, a_sb[:, b4, 0:128], ident[0:M, 0:M]
                    )
                    ata = atpool.tile([128, M], BF16, tag="ata_sb")
                    nc.scalar.copy(out=ata, in_=ata_ps)

                    # hw 68..195 at base partition 0; rows 0..59 (hw 68..127
                    # overlap with chunk A) are zeroed so they contribute 0.
                    atb_ps = ps_tr.tile([128, M], BF16, tag="ata")
                    nc.tensor.transpose(
                        atb_ps, a_sb[:, b4, 68:196], ident[0:M, 0:M]
                    )
                    atb = atpool.tile([128, M], BF16, tag="atb_sb")
                    nc.scalar.copy(out=atb, in_=atb_ps)
                    nc.vector.memset(atb[0:60, :], 0.0)

                    # einsum2: bapT[m, c] = sum_hw A[m,hw] * x[c,hw]
                    # All four 512-wide output chunks per A^T half so the
                    # PE loads each stationary operand once.
                    bap_ps = [
                        ps_bap.tile([M, 512], F32, tag="bap", name=f"bap_ps{nt}")
                        for nt in range(4)
                    ]
                    for nt in range(4):
                        nc.tensor.matmul(
                            bap_ps[nt],
                            lhsT=ata,
                            rhs=xta[:, 512 * nt : 512 * (nt + 1)],
                            start=True,
                            stop=False,
                        )
                    for nt in range(4):
                        nc.tensor.matmul(
                            bap_ps[nt],
                            lhsT=atb,
                            rhs=xtb[:, 512 * nt : 512 * (nt + 1)],
                            start=False,
                            stop=True,
                        )
                    for nt in range(4):
                        nc.vector.tensor_copy(
                            out=featsq[
                                32 * b4 : 32 * (b4 + 1),
                                512 * nt : 512 * (nt + 1),
                            ],
                            in_=bap_ps[nt],
                        )
                nc.gpsimd.dma_start(
                    out=feats.ap()[4 * q : 4 * q + 4].rearrange(
                        "b (m c) -> (b m) c", m=M
                    ),
                    in_=featsq,
                )
    nc.compile()
    return nc


def _build_phase2():
    """Per-core: featsT slice (partition-major, bf16) x WcT slice (bf16)
    -> partial [B, NCLS] (fp32)."""
    nc = _nc()
    ft = nc.dram_tensor("ft", [128, KT, B], BF16, kind="ExternalInput")
    wct = nc.dram_tensor("wct", [KPC, NCLS], BF16, kind="ExternalInput")
    part = nc.dram_tensor("part", [B, NCLS], F32, kind="ExternalOutput")

    KCH = 4  # k-tiles per DMA chunk

    with TileContext(nc) as tc:
        with (
            tc.tile_pool(name="fpool", bufs=1) as fpool,
            tc.tile_pool(name="wpool", bufs=8) as wpool,
            tc.tile_pool(name="opool", bufs=1) as opool,
            tc.tile_pool(name="ps_out", bufs=1, space="PSUM") as ps_out,
        ):
            ft_sb = fpool.tile([128, KT, B], BF16)
            nc.gpsimd.dma_start(out=ft_sb, in_=ft.ap())
            out_ps = ps_out.tile([B, NCLS], F32)
            for kc in range(KT // KCH):
                w_sb = wpool.tile([128, KCH, NCLS], BF16)
                eng = nc.scalar if kc % 2 else nc.sync
                eng.dma_start(
                    out=w_sb,
                    in_=wct.ap()[128 * KCH * kc : 128 * KCH * (kc + 1)].rearrange(
                        "(t p) n -> p t n", p=128
                    ),
                )
                for kl in range(KCH):
                    kt = kc * KCH + kl
                    nc.tensor.matmul(
                        out_ps,
                        lhsT=ft_sb[:, kt, :],
                        rhs=w_sb[:, kl, :],
                        start=(kt == 0),
                        stop=(kt == KT - 1),
                    )
            out_sb = opool.tile([B, NCLS], F32)
            nc.scalar.copy(out=out_sb, in_=out_ps)
            nc.sync.dma_start(out=part.ap(), in_=out_sb)
    nc.compile()
    return nc


def _install_ntff_hook():
    import types

    import trn_agent_boot.trn_boot as tb
    import concourse.bass_utils as bu

    hook = tb._ntff_profile_via_ctypes("/opt/axon/libaxon_pjrt.so")
    mod = types.ModuleType("antenv.axon_hooks")
    mod.get_axon_ntff_profile_hook = lambda: hook
    sys.modules["antenv.axon_hooks"] = mod
    bu.upload_artifacts = lambda tmpdir: "(skipped)"


def _run(nc, in_maps, label):
    core_ids = list(range(NCORES))
    if TRACE:
        _install_ntff_hook()
        res = run_bass_kernel_spmd(nc, in_maps, core_ids, trace=True)
        TRACE_INFO[label] = res.exec_time_ns
        TRACE_RES[label] = res
    else:
        res = run_bass_kernel_spmd(nc, in_maps, core_ids)
    return res.results


def kernel(x, Wa, ba, Wc, bc):
    import ml_dtypes

    bf16 = np.dtype(ml_dtypes.bfloat16)
    x = np.ascontiguousarray(x, dtype=np.float32).reshape(B, C, HW).astype(bf16)
    # wat[p, t, m] = Wa[m, p*CT + t] — matches the kernel's permuted c layout
    wat = np.ascontiguousarray(Wa.T, dtype=np.float32).astype(bf16).reshape(
        128, CT, M
    )
    ba2 = np.ascontiguousarray(ba, dtype=np.float32).reshape(M, 1)
    wct = np.ascontiguousarray(Wc.T, dtype=np.float32).astype(bf16)  # [KTOT, NCLS]

    if "p1" not in _cache:
        _cache["p1"] = _build_phase1()
    if "p2" not in _cache:
        _cache["p2"] = _build_phase2()

    in1 = [
        {"x": x[i * BPC : (i + 1) * BPC], "wat": wat, "ba": ba2}
        for i in range(NCORES)
    ]
    res1 = _run(_cache["p1"], in1, "phase1")
    feats = np.concatenate([r["feats"] for r in res1], axis=0)  # [B, KTOT] f32

    # ft[p, t, b] = feats[b, kslice + t*128 + p] (partition-major, bf16)
    featsT = np.ascontiguousarray(feats.T).astype(bf16)  # [KTOT, B]
    in2 = [
        {
            "ft": np.ascontiguousarray(
                featsT[i * KPC : (i + 1) * KPC].reshape(KT, 128, B).transpose(
                    1, 0, 2
                )
            ),
            "wct": wct[i * KPC : (i + 1) * KPC],
        }
        for i in range(NCORES)
    ]
    res2 = _run(_cache["p2"], in2, "phase2")
    parts = np.stack([r["part"] for r in res2], axis=0)  # [NCORES, B, NCLS]

    logits = parts.sum(axis=0) / float(HW) + np.asarray(bc, dtype=np.float32)
    return logits.astype(np.float32)
